# revision 1
# baseline (speedup 1.0000x reference)
"""AdaFocalLoss on 8 Trainium2 NeuronCores (Bass/Tile, SPMD).

Data-parallel over the batch axis, per the sharding hint: each core gets
8192 of the 65536 logit rows, the 15-entry gamma table is replicated, and
the per-core partial sums are combined on the host (the gather/unshard
step; the reduction over rows is order-independent).

Per-core kernel structure:
  - Rows are assigned to (slot, partition) SORTED BY TARGET on the host:
    slot s holds the 128 rows whose targets sit near the s-th quantile of
    the target distribution.  The row order is free to choose (the final
    loss is a sum over rows), and sorting makes the target-logit gather
    cheap: all 128 targets of a slot fall inside a static 64-column
    window around the slot's quantile center.
  - The shard streams as 64 contiguous 512 KB DMAs (one per slot).
  - ScalarE computes exp(x) for every element (fp16 out; the only engine
    with transcendentals).  The per-row sum of exps comes from the
    ACTIVATE's accum_out for 28 of the 64 slots and from a VectorE
    tensor_scalar cache-reduce over the exp tile for the other 36 - the
    split balances the two engines' busy time (~95 us each, right at the
    ~92 us HBM roofline for the 33 MB/core of logits).
  - The target logit x_t is gathered on VectorE in one pass per slot:
    scalar_tensor_tensor  (iota == target_p) * x  with accum_out, scanned
    only over the slot's 64-column window.
  - Tail per row ([128, 64] values): lse = ln(sumexp), logpt = x_t - lse,
    pt = exp(logpt); gamma's sign s and magnitude m are looked up via a
    telescoped sum_b(delta_b * [pt >= b/15]) computed with broadcast-AP
    tensor ops; loss = -(1 + eps - s*pt)^m * logpt via exp(m*ln(u)).
    The tail runs in two unequal parts (48/16 slots) so only the small
    final part is exposed past the stream.
  - Per-partition row sums are reduced across partitions with a PE
    matmul against a ones vector; each core emits one f32 scalar.

The gather windows are data-independent quantile bands (+-32 columns
~ 5.8 sigma of the sampling deviation for iid targets; the reference
distribution measures a max deviation of 26).  If an unusual
target distribution ever falls outside them, the host check catches it
and the kernel transparently rebuilds with full-width windows (slower
but always correct).
"""

import sys

for _p in ("/opt/trn_rl_repo",):
    if _p not in sys.path:
        sys.path.insert(0, _p)

import numpy as np

NUM_BINS = 15
EPS = 1e-20
N, C = 65536, 1000
NCORES = 8
NSHARD = N // NCORES  # 8192 rows per core
P = 128  # SBUF partitions
R = NSHARD // P  # 64 row-slots per partition
W = 64  # gather window width (columns) per row-slot
TAIL_BOUNDS = [0, 48, 64]  # unequal tail parts: only the small last one is exposed
NPART = len(TAIL_BOUNDS) - 1
ACT_ACC = 28  # row-sums accumulated on ScalarE (the rest go to VectorE)
PAIRED = False  # pairing measured slower: wide ACTIVATEs run at a worse per-element rate
EO_F16 = True  # dtype of the exp scratch tile
EARLY = 2  # row-slots prefetched ahead of the constant loads


def _slot_lo(w):
    # static window starts: slot s is centered on the s-th target quantile
    return [min(max(int(C * (s + 0.5) / R) - w // 2, 0), C - w) for s in range(R)]


def _split_excess_waits(nc, mybir, max_waits=1):
    """This container's walrus supports only one sync-wait command per
    instruction; hoist extra waits onto preceding same-engine no-ops."""
    ctr = 0
    for f in nc.m.functions:
        for bb in f.blocks:
            new_insts = []
            changed = False
            for inst in bb.instructions:
                si = inst.sync_info
                if si is not None and si.on_wait and len(si.on_wait) > max_waits:
                    waits = list(si.on_wait)
                    excess, keep = waits[:-max_waits], waits[-max_waits:]
                    for i in range(0, len(excess), max_waits):
                        ctr += 1
                        new_insts.append(
                            mybir.InstNoOp(
                                name=f"I-waitsplit-{ctr}",
                                sync_info=mybir.SyncInfo(
                                    on_wait=list(excess[i : i + max_waits]),
                                    on_update=[],
                                ),
                                bass_nofuse=True,
                                engine=inst.engine,
                            )
                        )
                    si.on_wait = keep
                    changed = True
                new_insts.append(inst)
            if changed:
                bb.instructions[:] = new_insts


def _build(w):
    import concourse.bass as bass
    import concourse.tile as tile
    from concourse import mybir

    f32 = mybir.dt.float32
    f16 = mybir.dt.float16 if EO_F16 else mybir.dt.float32
    AF = mybir.ActivationFunctionType
    ALU = mybir.AluOpType
    NB = NUM_BINS
    slot_lo = _slot_lo(w)

    nc = bass.Bass()
    x = nc.declare_dram_parameter("x", [NSHARD, C], f32, isOutput=False)
    tmap = nc.declare_dram_parameter("tmap", [P, R], f32, isOutput=False)
    iota = nc.declare_dram_parameter("iota", [P, C], f32, isOutput=False)
    gb = nc.declare_dram_parameter("gb", [P, NB], f32, isOutput=False)
    out = nc.declare_dram_parameter("out", [1, 1], f32, isOutput=True)

    # target-sorted rank-major layout: HBM row s*128 + p holds the row for
    # slot s, partition p, so each slot is one contiguous 512 KB DMA
    x3 = x[:].rearrange("(s p) c -> s p c", s=R, p=P)
    # paired view: u-th pair = slots (2u, 2u+1) in one [P, 2, C] transfer
    x4 = x[:].rearrange("(u q p) c -> u p q c", u=R // 2, q=2, p=P)

    # slots whose row-sum of exps is accumulated on ScalarE (cheap marginal
    # cost) vs VectorE (ts cache-reduce), spread evenly for smooth overlap
    if PAIRED:
        # groups of 4: [A A D D] x14 then [D D D D] x2 -> 28 ACT slots and
        # 18 adjacent D-pairs that share one DMA and one wide ACTIVATE
        act_slots = set()
        for g in range(R // 4):
            if g < 14:
                act_slots.add(4 * g)
                act_slots.add(4 * g + 1)
        schedule = []
        for g in range(R // 4):
            base = 4 * g
            if g < 14:
                schedule += [("A", base), ("A", base + 1), ("D2", base + 2)]
            else:
                schedule += [("D2", base), ("D2", base + 2)]
    else:
        act_slots = set(
            s for s in range(R) if (s * ACT_ACC) // R != ((s + 1) * ACT_ACC) // R
        )
        schedule = [("A" if s in act_slots else "D", s) for s in range(R)]

    def slot_part(slot):
        h = 0
        while slot >= TAIL_BOUNDS[h + 1]:
            h += 1
        return h, slot - TAIL_BOUNDS[h]

    part_w = [TAIL_BOUNDS[h + 1] - TAIL_BOUNDS[h] for h in range(NPART)]

    with tile.TileContext(nc) as tc:
        with (
            tc.tile_pool(name="const", bufs=1) as cpool,
            tc.tile_pool(name="io", bufs=8) as iopool,
            tc.tile_pool(name="escr", bufs=3) as epool,
            tc.tile_pool(name="sscr", bufs=3) as spool,
            tc.tile_pool(name="acc", bufs=1) as apool,
            tc.tile_pool(name="tail", bufs=3) as tpool,
            tc.tile_pool(name="psum", bufs=1, space="PSUM") as ppool,
        ):
            # a few row-slots stream before the constant loads so compute
            # can begin immediately
            early = {}
            for s in range(EARLY):
                et = iopool.tile([P, C], f32, tag="xtile", name=f"xtile_e{s}")
                nc.sync.dma_start(et[:], x3[s, :, :])
                early[s] = et

            iota_t = cpool.tile([P, C], f32, tag="iota")
            nc.sync.dma_start(iota_t[:], iota[:])
            tmap_t = cpool.tile([P, R], f32, tag="tmap")
            nc.sync.dma_start(tmap_t[:], tmap[:])
            gb_t = cpool.tile([P, NB], f32, tag="gb")
            nc.sync.dma_start(gb_t[:], gb[:])

            # gamma sign/magnitude tables and their telescoped deltas:
            # g(bin(pt)) = sum_b dg_b * [pt >= b/15]
            sgn = cpool.tile([P, NB], f32, tag="sgn")
            nc.scalar.activation(sgn[:], gb_t[:], AF.Sign)
            mag = cpool.tile([P, NB], f32, tag="mag")
            nc.scalar.activation(mag[:], gb_t[:], AF.Abs)
            ds = cpool.tile([P, NB], f32, tag="ds")
            nc.vector.tensor_copy(ds[:, 0:1], sgn[:, 0:1])
            nc.vector.tensor_sub(ds[:, 1:NB], sgn[:, 1:NB], sgn[:, 0 : NB - 1])
            dm = cpool.tile([P, NB], f32, tag="dm")
            nc.vector.tensor_copy(dm[:, 0:1], mag[:, 0:1])
            nc.vector.tensor_sub(dm[:, 1:NB], mag[:, 1:NB], mag[:, 0 : NB - 1])
            # bin thresholds b/15, derived from the iota constant
            thr = cpool.tile([P, NB], f32, tag="thr")
            nc.vector.tensor_scalar(
                thr[:], iota_t[:, 0:NB], 1.0 / NB, None, ALU.mult
            )

            # per-half accumulators so each tail half only depends on its
            # own half of the main loop
            sumexp = [
                apool.tile([P, part_w[h]], f32, tag=f"sumexp{h}", name=f"sumexp{h}")
                for h in range(NPART)
            ]
            xt = [
                apool.tile([P, part_w[h]], f32, tag=f"xt{h}", name=f"xt{h}")
                for h in range(NPART)
            ]
            rowsums = []

            def tail_half(h):
                se, xh = sumexp[h], xt[h]
                F = part_w[h]
                lse = tpool.tile([P, F], f32, tag="lse")
                nc.scalar.activation(lse[:], se[:], AF.Ln)
                logpt = tpool.tile([P, F], f32, tag="logpt")
                nc.vector.tensor_sub(logpt[:], xh[:], lse[:])
                pt = tpool.tile([P, F], f32, tag="pt")
                nc.scalar.activation(pt[:], logpt[:], AF.Exp)

                # s(pt), m(pt) via broadcast APs: ge[p,j,b] = pt[p,j]>=thr[p,b]
                ge = tpool.tile([P, F * NB], f32, tag="ge")
                ge3 = ge[:].rearrange("p (f b) -> p f b", b=NB)
                pt_b = (
                    pt[:]
                    .rearrange("p (f one) -> p f one", one=1)
                    .broadcast_to([P, F, NB])
                )
                thr_b = (
                    thr[:]
                    .rearrange("p (one b) -> p one b", one=1)
                    .broadcast_to([P, F, NB])
                )
                nc.vector.tensor_tensor(ge3, pt_b, thr_b, ALU.is_ge)
                ds_b = (
                    ds[:]
                    .rearrange("p (one b) -> p one b", one=1)
                    .broadcast_to([P, F, NB])
                )
                dm_b = (
                    dm[:]
                    .rearrange("p (one b) -> p one b", one=1)
                    .broadcast_to([P, F, NB])
                )
                prods = tpool.tile([P, F * NB], f32, tag="prods")
                nc.vector.tensor_tensor(
                    prods[:].rearrange("p (f b) -> p f b", b=NB), ge3, ds_b, ALU.mult
                )
                s_acc = tpool.tile([P, F], f32, tag="s_acc")
                nc.vector.tensor_reduce(
                    s_acc[:], prods[:].rearrange("p (f b) -> p f b", b=NB),
                    mybir.AxisListType.X, ALU.add,
                )
                prodm = tpool.tile([P, F * NB], f32, tag="prodm")
                nc.vector.tensor_tensor(
                    prodm[:].rearrange("p (f b) -> p f b", b=NB), ge3, dm_b, ALU.mult
                )
                m_acc = tpool.tile([P, F], f32, tag="m_acc")
                nc.vector.tensor_reduce(
                    m_acc[:], prodm[:].rearrange("p (f b) -> p f b", b=NB),
                    mybir.AxisListType.X, ALU.add,
                )

                # u = 1 + eps - s*pt ;  y = u^m = exp(m * ln(u))
                nspt = tpool.tile([P, F], f32, tag="nspt")
                nc.vector.scalar_tensor_tensor(
                    nspt[:], s_acc[:], -1.0, pt[:], ALU.mult, ALU.mult
                )
                u = tpool.tile([P, F], f32, tag="u")
                nc.vector.tensor_scalar(u[:], nspt[:], 1.0 + EPS, None, ALU.add)
                v = tpool.tile([P, F], f32, tag="v")
                nc.scalar.activation(v[:], u[:], AF.Ln)
                w_ = tpool.tile([P, F], f32, tag="w")
                nc.vector.tensor_mul(w_[:], v[:], m_acc[:])
                y = tpool.tile([P, F], f32, tag="y")
                nc.scalar.activation(y[:], w_[:], AF.Exp)

                # per-partition partial of sum_j y*logpt (negated on host)
                prod = tpool.tile([P, F], f32, tag="prod")
                nc.vector.tensor_mul(prod[:], y[:], logpt[:])
                rs = tpool.tile([P, 1], f32, tag=f"rowsum{h}", name=f"rowsum{h}")
                nc.vector.tensor_reduce(
                    rs[:], prod[:], mybir.AxisListType.X, ALU.add
                )
                rowsums.append(rs)

            def do_slot(slot, xtile, off, is_act, eo=None):
                h, col = slot_part(slot)
                if not is_act:
                    edum = epool.tile([P, C], f16, tag="edum")
                    nc.vector.tensor_scalar(
                        edum[:], eo[:], 1.0, None,
                        ALU.mult, ALU.add,
                        accum_out=sumexp[h][:, col : col + 1],
                    )
                # rows are target-sorted, so this slot's targets all sit
                # inside a static window: the gather scans only it
                lo = slot_lo[slot]
                so = spool.tile([P, w], f32, tag="so")
                nc.vector.scalar_tensor_tensor(
                    so[:],
                    iota_t[:, lo : lo + w],
                    tmap_t[:, slot : slot + 1],
                    xtile[:, off + lo : off + lo + w],
                    ALU.is_equal,
                    ALU.mult,
                    accum_out=xt[h][:, col : col + 1],
                )

            done_halves = set()
            for kind, slot in schedule:
                if kind in ("A", "D"):
                    if slot in early:
                        xtile = early[slot]
                    else:
                        xtile = iopool.tile([P, C], f32, tag="xtile")
                        nc.sync.dma_start(xtile[:], x3[slot, :, :])
                    eo = epool.tile([P, C], f16, tag="eo")
                    if kind == "A":
                        h, col = slot_part(slot)
                        nc.scalar.activation(
                            eo[:], xtile[:], AF.Exp,
                            accum_out=sumexp[h][:, col : col + 1],
                        )
                        do_slot(slot, xtile, 0, True)
                    else:
                        nc.scalar.activation(eo[:], xtile[:], AF.Exp)
                        do_slot(slot, xtile, 0, False, eo=eo)
                    hi = slot
                else:  # D2: slots (slot, slot+1) in one DMA + one ACTIVATE
                    xtile = iopool.tile([P, 2 * C], f32, tag="xtile2")
                    nc.sync.dma_start(
                        xtile[:].rearrange("p (q c) -> p q c", q=2),
                        x4[slot // 2, :, :, :],
                    )
                    eo2 = epool.tile([P, 2 * C], f16, tag="eo2")
                    nc.scalar.activation(eo2[:], xtile[:], AF.Exp)
                    for q in range(2):
                        s2 = slot + q
                        h, col = slot_part(s2)
                        edum = epool.tile([P, C], f16, tag="edum")
                        nc.vector.tensor_scalar(
                            edum[:], eo2[:, q * C : (q + 1) * C], 1.0, None,
                            ALU.mult, ALU.add,
                            accum_out=sumexp[h][:, col : col + 1],
                        )
                        lo = slot_lo[s2]
                        so = spool.tile([P, w], f32, tag="so")
                        nc.vector.scalar_tensor_tensor(
                            so[:],
                            iota_t[:, lo : lo + w],
                            tmap_t[:, s2 : s2 + 1],
                            xtile[:, q * C + lo : q * C + lo + w],
                            ALU.is_equal,
                            ALU.mult,
                            accum_out=xt[h][:, col : col + 1],
                        )
                    hi = slot + 1
                for hh in range(NPART - 1):
                    if hi >= TAIL_BOUNDS[hh + 1] - 1 and hh not in done_halves:
                        done_halves.add(hh)
                        tail_half(hh)  # overlaps the rest of the stream
            tail_half(NPART - 1)

            total = rowsums[0]
            for q in range(1, NPART):
                tq = tpool.tile([P, 1], f32, tag=f"tq{q}", name=f"tq{q}")
                nc.vector.tensor_add(tq[:], total[:], rowsums[q][:])
                total = tq
            ones = tpool.tile([P, 1], f32, tag="ones")
            nc.vector.memset(ones[:], 1.0)
            ps = ppool.tile([1, 1], f32, tag="ps")
            nc.tensor.matmul(ps[:], ones[:], total[:], start=True, stop=True)
            res = tpool.tile([1, 1], f32, tag="res")
            nc.scalar.copy(res[:], ps[:])
            nc.sync.dma_start(out[:], res[:])

    _split_excess_waits(nc, mybir, max_waits=1)
    return nc


_NC_CACHE = {}


def _get_nc(w):
    if w not in _NC_CACHE:
        _NC_CACHE[w] = _build(w)
    return _NC_CACHE[w]


def _make_in_maps(input, target, gammas, w):
    inp = np.ascontiguousarray(np.asarray(input, dtype=np.float32))
    tgt = np.asarray(target).astype(np.int64)
    gam = np.asarray(gammas, dtype=np.float32)
    assert inp.shape == (N, C) and tgt.shape == (N,) and gam.shape == (NUM_BINS,)

    iota_const = np.ascontiguousarray(
        np.broadcast_to(np.arange(C, dtype=np.float32), (P, C))
    )
    gb_const = np.ascontiguousarray(np.broadcast_to(gam, (P, NUM_BINS)))
    slot_lo = np.asarray(_slot_lo(w), dtype=np.int64)

    in_maps = []
    for i in range(NCORES):
        tshard = tgt[NSHARD * i : NSHARD * (i + 1)]
        # sort rows by target; rank r -> slot r//P, partition r%P, so each
        # slot's 128 targets fall inside its static gather window
        order = np.argsort(tshard, kind="stable")
        tsorted = tshard[order]
        by_slot = tsorted.reshape(R, P)  # [slot, partition]
        lo = slot_lo[:, None]
        if not np.all((by_slot >= lo) & (by_slot <= lo + (w - 1))):
            return None  # caller falls back to full-width windows
        shard = np.ascontiguousarray(inp[NSHARD * i : NSHARD * (i + 1)][order])
        tmap = np.ascontiguousarray(by_slot.T).astype(np.float32)  # [P, R]
        in_maps.append(
            {"x": shard, "tmap": tmap, "iota": iota_const, "gb": gb_const}
        )
    return in_maps


def kernel(input, target, gammas, _trace=False, _tmpdir=None):
    from concourse.bass_utils import run_bass_kernel_spmd

    in_maps = _make_in_maps(input, target, gammas, W)
    w = W
    if in_maps is None:
        # pathological target distribution: use full-width gather windows
        w = C
        in_maps = _make_in_maps(input, target, gammas, w)
        assert in_maps is not None  # w == C always satisfies the window check

    res = run_bass_kernel_spmd(
        _get_nc(w),
        in_maps,
        core_ids=list(range(NCORES)),
        trace=_trace,
        tmpdir=_tmpdir,
    )
    partials = [float(res.results[i]["out"][0, 0]) for i in range(NCORES)]
    total = -np.float32(np.sum(np.asarray(partials, dtype=np.float32)))
    if _trace:
        kernel._last_result = res
    return np.array(total, dtype=np.float32)



# revision 6
# speedup vs baseline: 1.0881x; 1.0881x over previous
"""AdaFocalLoss on 8 Trainium2 NeuronCores (Bass/Tile, SPMD).

Data-parallel over the batch axis: each core gets 8192 of the 65536
logit rows, the 15-entry gamma table is replicated, and the per-core
per-slot partial sums are combined on the host (the reduction over rows
is order-independent).

Per-core kernel structure (v2 — DMA-chunked):
  - Rows are assigned to (slot, partition) SORTED BY TARGET on the host:
    slot s holds the 128 rows whose targets sit near the s-th quantile of
    the target distribution, so all 128 targets of a slot fall inside a
    static 64-column window around the slot's quantile center.
  - The stream is issued as multi-slot chunk DMAs with a partition-major
    host layout, so each partition line is one 4*k*1000-byte descriptor
    (k slots/chunk).  Bigger descriptors cut the per-packet SDMA bus
    overhead (4KB lines run at ~345 GB/s; 16KB lines closer to the ~358
    GB/s HBM-per-core limit).  The chunk sizes taper: singles at the
    head (compute starts early), quads in steady state, and the final
    slot is two half-column DMAs so only ~0.6us of exp is exposed after
    the last byte lands.
  - ScalarE computes exp(x) for every element (fp16 out; the only
    engine with transcendentals).  The per-row sum of exps comes from
    the ACTIVATE's accum_out for 40 of the 64 slots and from a VectorE
    tensor_reduce over the exp tile for the other 24 — ScalarE ~87us
    and VectorE ~58us busy against the ~92us stream.
  - The target logit x_t is gathered on VectorE in one pass per slot:
    scalar_tensor_tensor (iota == (t - lo_s)) * x with accum_out over
    the slot's static 64-column window; the window iota and the
    per-slot-adjusted targets are host-precomputed constants (one small
    DMA — no [128,1000] iota constant).
  - Tail per row: lse = ln(sumexp), logpt = x_t - lse, pt = exp(logpt);
    gamma's sign s and magnitude m come from ONE fused telescope pass
    (ge = pt >= thr over a packed [ds|dm] delta table, multiply, reduce)
    instead of two; loss = -(1 + eps - s*pt)^m * logpt with the (1+eps,
    -1) affine folded into the Ln ACTIVATE.  The tail runs in four
    parts (32/24/7/1 slots) so only the 1-slot final part is exposed
    past the stream.
  - Each part writes its per-partition loss products into one column
    range of a [128, 64] tile; a single PE matmul against a ones vector
    reduces partitions, and the host sums the resulting [1, 64] f32
    outputs across cores (and negates).

The gather windows are data-independent quantile bands (+-32 columns
~ 5.8 sigma of the sampling deviation for iid targets; the reference
distribution measures a max deviation of 26).  If an unusual target
distribution ever falls outside them, the host check catches it and the
kernel transparently rebuilds with full-width windows (slower but
always correct).
"""

import sys

for _p in ("/opt/trn_rl_repo",):
    if _p not in sys.path:
        sys.path.insert(0, _p)

import numpy as np

NUM_BINS = 15
EPS = 1e-20
N, C = 65536, 1000
NCORES = 8
NSHARD = N // NCORES  # 8192 rows per core
P = 128  # SBUF partitions
R = NSHARD // P  # 64 row-slots per partition
W = 64  # gather window width (columns) per row-slot
ACT_SLOTS = 40  # row-sums accumulated on ScalarE (the rest on VectorE)
HALF_COL = 500  # column split point of the final slot's two DMAs


def _slot_lo(w):
    # static window starts: slot s is centered on the s-th target quantile
    return [min(max(int(C * (s + 0.5) / R) - w // 2, 0), C - w) for s in range(R)]


def _chunk_plan(w):
    """[(base_slot, n_slots), ...] + whether the last slot is halved.

    Multi-slot chunks need the partition-major host layout; the final
    halved slot overlaps its exp with its own DMA.  The full-width
    fallback (w == C) keeps every slot a single chunk because slot 63's
    gather window spans both halves there.
    """
    if w == C:
        return [(s, 1) for s in range(R)], False
    chunks = [(0, 1), (1, 1), (2, 2), (4, 4)]
    chunks += [(8 + 4 * i, 4) for i in range(12)]  # slots 8..55
    chunks += [(56, 2), (58, 2), (60, 2), (62, 1)]
    return chunks, True  # slot 63 in two half-column DMAs


# tail parts: only the small final part is exposed past the stream
TAIL_BOUNDS = [0, 32, 56, 63, 64]
NPART = len(TAIL_BOUNDS) - 1


def _split_excess_waits(nc, mybir, max_waits=1):
    """This container's walrus supports only one sync-wait command per
    instruction; hoist extra waits onto preceding same-engine no-ops."""
    ctr = 0
    for f in nc.m.functions:
        for bb in f.blocks:
            new_insts = []
            changed = False
            for inst in bb.instructions:
                si = inst.sync_info
                if si is not None and si.on_wait and len(si.on_wait) > max_waits:
                    waits = list(si.on_wait)
                    excess, keep = waits[:-max_waits], waits[-max_waits:]
                    for i in range(0, len(excess), max_waits):
                        ctr += 1
                        new_insts.append(
                            mybir.InstNoOp(
                                name=f"I-waitsplit-{ctr}",
                                sync_info=mybir.SyncInfo(
                                    on_wait=list(excess[i : i + max_waits]),
                                    on_update=[],
                                ),
                                bass_nofuse=True,
                                engine=inst.engine,
                            )
                        )
                    si.on_wait = keep
                    changed = True
                new_insts.append(inst)
            if changed:
                bb.instructions[:] = new_insts


def _build(w):
    import concourse.bass as bass
    import concourse.tile as tile
    from concourse import mybir

    f32 = mybir.dt.float32
    f16 = mybir.dt.float16
    AF = mybir.ActivationFunctionType
    ALU = mybir.AluOpType
    X = mybir.AxisListType.X
    NB = NUM_BINS
    slot_lo = _slot_lo(w)
    chunks, halved = _chunk_plan(w)

    nc = bass.Bass()
    x = nc.declare_dram_parameter("x", [NSHARD, C], f32, isOutput=False)
    # packed small constants: [tmap_adj (R) | gammas (NB) | iota (w)]
    tgb = nc.declare_dram_parameter("tgb", [P, R + NB + w], f32, isOutput=False)
    out = nc.declare_dram_parameter("out", [1, R], f32, isOutput=True)

    x_ap = x[:]

    # slots whose row-sum of exps is accumulated on ScalarE vs VectorE;
    # the late slots are all ScalarE so the exposed tail path is short
    n_d = R - ACT_SLOTS
    d_lim = 48 if w != C else R
    act_slots = set(range(R)) - set(
        s for s in range(d_lim) if (s * n_d) // d_lim != ((s + 1) * n_d) // d_lim
    )

    def slot_part(slot):
        h = 0
        while slot >= TAIL_BOUNDS[h + 1]:
            h += 1
        return h, slot - TAIL_BOUNDS[h]

    part_w = [TAIL_BOUNDS[h + 1] - TAIL_BOUNDS[h] for h in range(NPART)]

    with tile.TileContext(nc) as tc:
        with (
            tc.tile_pool(name="const", bufs=1) as cpool,
            tc.tile_pool(name="io", bufs=1) as iopool,
            tc.tile_pool(name="escr", bufs=3) as epool,
            tc.tile_pool(name="sscr", bufs=3) as spool,
            tc.tile_pool(name="acc", bufs=1) as apool,
            tc.tile_pool(name="tail", bufs=2) as tpool,
            tc.tile_pool(name="psum", bufs=1, space="PSUM") as ppool,
        ):
            # the first two chunks stream before the constant load so
            # compute can begin immediately
            def chunk_dma(ci):
                s0, k = chunks[ci]
                xt = iopool.tile(
                    [P, k * C], f32, tag=f"xt{k}", name=f"xtile_c{ci}", bufs=3 if k <= 1 else (3 if k == 4 else 2)
                )
                src = x_ap[s0 * P : (s0 + k) * P, :].rearrange(
                    "(p k) c -> p (k c)", p=P, k=k
                )
                nc.sync.dma_start(xt[:], src)
                return xt

            early = {ci: chunk_dma(ci) for ci in range(2)}

            tgb_t = cpool.tile([P, R + NB + w], f32, tag="tgb")
            nc.sync.dma_start(tgb_t[:], tgb[:])
            tmap_t = tgb_t[:, 0:R]
            gb_t = tgb_t[:, R : R + NB]
            iota_t = tgb_t[:, R + NB : R + NB + w]

            dsm = cpool.tile([P, 2 * NB], f32, tag="dsm")
            thr = cpool.tile([P, NB], f32, tag="thr")
            ones = cpool.tile([P, 1], f32, tag="ones")

            def derive_consts():
                # gamma sign/magnitude tables, telescoped into one packed
                # [ds | dm] delta table: g(bin(pt)) = sum_b dg_b*[pt>=b/15]
                sgn = cpool.tile([P, NB], f32, tag="sgn")
                nc.scalar.activation(sgn[:], gb_t, AF.Sign)
                mag = cpool.tile([P, NB], f32, tag="mag")
                nc.scalar.activation(mag[:], gb_t, AF.Abs)
                nc.vector.tensor_copy(dsm[:, 0:1], sgn[:, 0:1])
                nc.vector.tensor_sub(dsm[:, 1:NB], sgn[:, 1:NB], sgn[:, 0 : NB - 1])
                nc.vector.tensor_copy(dsm[:, NB : NB + 1], mag[:, 0:1])
                nc.vector.tensor_sub(
                    dsm[:, NB + 1 : 2 * NB], mag[:, 1:NB], mag[:, 0 : NB - 1]
                )
                # bin thresholds b/15, derived from the iota constant
                nc.vector.tensor_scalar(
                    thr[:], iota_t[:, 0:NB], 1.0 / NB, None, ALU.mult
                )
                nc.vector.memset(ones[:], 1.0)

            # per-part accumulators; the final part has two sumexp
            # columns when the last slot streams as two half DMAs
            se_w = list(part_w)
            if halved:
                se_w[-1] = 2
            sumexp = [
                apool.tile([P, se_w[h]], f32, tag=f"sumexp{h}", name=f"sumexp{h}")
                for h in range(NPART)
            ]
            xt_acc = [
                apool.tile([P, part_w[h]], f32, tag=f"xt{h}", name=f"xta{h}")
                for h in range(NPART)
            ]
            # per-slot per-partition loss products; one PE matmul reduces
            # partitions at the very end
            prodcat = apool.tile([P, R], f32, tag="prodcat")

            def gather(slot, xtile, off):
                h, col = slot_part(slot)
                lo = slot_lo[slot]
                so = spool.tile([P, w], f32, tag="so")
                nc.vector.scalar_tensor_tensor(
                    so[:],
                    iota_t,
                    tmap_t[:, slot : slot + 1],
                    xtile[:, off + lo : off + lo + w],
                    ALU.is_equal,
                    ALU.mult,
                    accum_out=xt_acc[h][:, col : col + 1],
                )

            def do_slot(slot, xtile, off):
                h, col = slot_part(slot)
                eo = epool.tile([P, C], f16, tag="eo")
                if slot in act_slots:
                    nc.scalar.activation(
                        eo[:],
                        xtile[:, off : off + C],
                        AF.Exp,
                        accum_out=sumexp[h][:, col : col + 1],
                    )
                else:
                    nc.scalar.activation(eo[:], xtile[:, off : off + C], AF.Exp)
                    nc.vector.tensor_reduce(
                        sumexp[h][:, col : col + 1], eo[:], X, ALU.add
                    )
                gather(slot, xtile, off)

            def tail_part(h):
                F = part_w[h]
                se = sumexp[h]
                if se_w[h] != F:  # halved final slot: combine the two sums
                    se2 = tpool.tile([P, 1], f32, tag="se2", name="se2")
                    nc.vector.tensor_add(se2[:], se[:, 0:1], se[:, 1:2])
                    se = se2
                lse = tpool.tile([P, F], f32, tag="lse")
                nc.scalar.activation(lse[:], se[:], AF.Ln)
                logpt = tpool.tile([P, F], f32, tag="logpt")
                nc.vector.tensor_sub(logpt[:], xt_acc[h][:], lse[:])
                pt = tpool.tile([P, F], f32, tag="pt")
                nc.scalar.activation(pt[:], logpt[:], AF.Exp)

                # fused telescope: ge[p,f,j,b] = pt[p,f] >= thr[p,b],
                # prods = ge * [ds|dm][p,j,b], reduce b -> sm[p,f,j]
                ge = tpool.tile([P, F * 2 * NB], f32, tag="ge")
                ge4 = ge[:].rearrange("p (f j b) -> p f j b", j=2, b=NB)
                pt_b = (
                    pt[:]
                    .rearrange("p (f j b) -> p f j b", j=1, b=1)
                    .broadcast_to([P, F, 2, NB])
                )
                thr_b = (
                    thr[:]
                    .rearrange("p (f j b) -> p f j b", f=1, j=1)
                    .broadcast_to([P, F, 2, NB])
                )
                nc.vector.tensor_tensor(ge4, pt_b, thr_b, ALU.is_ge)
                dsm_b = (
                    dsm[:]
                    .rearrange("p (f j b) -> p f j b", f=1, j=2)
                    .broadcast_to([P, F, 2, NB])
                )
                prods = tpool.tile([P, F * 2 * NB], f32, tag="prods")
                nc.vector.tensor_tensor(
                    prods[:].rearrange("p (f j b) -> p f j b", j=2, b=NB),
                    ge4,
                    dsm_b,
                    ALU.mult,
                )
                sm = tpool.tile([P, F * 2], f32, tag="sm")
                nc.vector.tensor_reduce(
                    sm[:], prods[:].rearrange("p (f j b) -> p f j b", j=2, b=NB),
                    X, ALU.add,
                )
                sm3 = sm[:].rearrange("p (f j) -> p f j", j=2)
                pt3 = pt[:].rearrange("p (f j) -> p f j", j=1)

                # u = 1 + eps - s*pt ; y = u^m = exp(m * ln(u)); the
                # (scale=-1, bias=1+eps) affine rides the Ln ACTIVATE
                spt = tpool.tile([P, F], f32, tag="spt")
                nc.vector.tensor_mul(
                    spt[:].rearrange("p (f j) -> p f j", j=1), sm3[:, :, 0:1], pt3
                )
                v = tpool.tile([P, F], f32, tag="v")
                nc.scalar.activation(v[:], spt[:], AF.Ln, bias=1.0 + EPS, scale=-1.0)
                w_ = tpool.tile([P, F], f32, tag="w")
                nc.vector.tensor_mul(
                    w_[:].rearrange("p (f j) -> p f j", j=1),
                    v[:].rearrange("p (f j) -> p f j", j=1),
                    sm3[:, :, 1:2],
                )
                y = tpool.tile([P, F], f32, tag="y")
                nc.scalar.activation(y[:], w_[:], AF.Exp)
                # per-slot per-partition partial of sum y*logpt (negated
                # on host)
                off = TAIL_BOUNDS[h]
                nc.vector.tensor_mul(prodcat[:, off : off + F], y[:], logpt[:])

            done_parts = set()
            for ci, (s0, k) in enumerate(chunks):
                xtile = early[ci] if ci in early else chunk_dma(ci)
                for j in range(k):
                    do_slot(s0 + j, xtile, j * C)
                if ci == 1:
                    # after the first two exps are queued, so the
                    # constant ACTIVATEs don't delay them on ScalarE
                    derive_consts()
                hi = s0 + k - 1
                for h in range(NPART - 1):
                    if hi >= TAIL_BOUNDS[h + 1] - 1 and h not in done_parts:
                        done_parts.add(h)
                        tail_part(h)  # overlaps the rest of the stream

            if halved:
                # final slot streams as two half-column DMAs: the first
                # half's exp overlaps the second half's transfer
                slot = R - 1
                lo = slot_lo[slot]
                for q, (c0, c1) in enumerate([(0, HALF_COL), (HALF_COL, C)]):
                    ht = iopool.tile(
                        [P, c1 - c0], f32, tag="xth", name=f"xth{q}", bufs=2
                    )
                    nc.sync.dma_start(ht[:], x_ap[slot * P : (slot + 1) * P, c0:c1])
                    eo = epool.tile([P, c1 - c0], f16, tag="eoh")
                    nc.scalar.activation(
                        eo[:], ht[:], AF.Exp,
                        accum_out=sumexp[NPART - 1][:, q : q + 1],
                    )
                    if c0 <= lo and lo + w <= c1:
                        so = spool.tile([P, w], f32, tag="so")
                        nc.vector.scalar_tensor_tensor(
                            so[:],
                            iota_t,
                            tmap_t[:, slot : slot + 1],
                            ht[:, lo - c0 : lo - c0 + w],
                            ALU.is_equal,
                            ALU.mult,
                            accum_out=xt_acc[NPART - 1][:, 0:1],
                        )
            tail_part(NPART - 1)

            ps = ppool.tile([1, R], f32, tag="ps")
            nc.tensor.matmul(ps[:], ones[:], prodcat[:], start=True, stop=True)
            res = tpool.tile([1, R], f32, tag="res")
            nc.scalar.copy(res[:], ps[:])
            nc.sync.dma_start(out[:], res[:])

    _split_excess_waits(nc, mybir, max_waits=1)
    return nc


_NC_CACHE = {}


def _get_nc(w):
    if w not in _NC_CACHE:
        _NC_CACHE[w] = _build(w)
    return _NC_CACHE[w]


def _make_in_maps(input, target, gammas, w):
    inp = np.ascontiguousarray(np.asarray(input, dtype=np.float32))
    tgt = np.asarray(target).astype(np.int64)
    gam = np.asarray(gammas, dtype=np.float32)
    assert inp.shape == (N, C) and tgt.shape == (N,) and gam.shape == (NUM_BINS,)

    slot_lo = np.asarray(_slot_lo(w), dtype=np.int64)
    chunks, _halved = _chunk_plan(w)

    in_maps = []
    for i in range(NCORES):
        tshard = tgt[NSHARD * i : NSHARD * (i + 1)]
        # sort rows by target; rank r -> slot r//P, partition r%P, so each
        # slot's 128 targets fall inside its static gather window
        order = np.argsort(tshard, kind="stable")
        tsorted = tshard[order]
        by_slot = tsorted.reshape(R, P)  # [slot, partition]
        lo = slot_lo[:, None]
        if not np.all((by_slot >= lo) & (by_slot <= lo + (w - 1))):
            return None  # caller falls back to full-width windows
        shard = inp[NSHARD * i : NSHARD * (i + 1)][order]
        # chunk layout: within a k-slot chunk, partition-major so each
        # partition line is one contiguous 4*k*C-byte DMA descriptor
        xbuf = np.empty_like(shard)
        for s0, k in chunks:
            blk = shard[s0 * P : (s0 + k) * P]
            if k > 1:
                blk = (
                    blk.reshape(k, P, C).transpose(1, 0, 2).reshape(k * P, C)
                )
            xbuf[s0 * P : (s0 + k) * P] = blk
        tmap_adj = (by_slot - lo).T.astype(np.float32)  # [P, R]
        tgb = np.concatenate(
            [
                tmap_adj,
                np.broadcast_to(gam, (P, NUM_BINS)),
                np.broadcast_to(np.arange(w, dtype=np.float32), (P, w)),
            ],
            axis=1,
        )
        in_maps.append(
            {"x": np.ascontiguousarray(xbuf), "tgb": np.ascontiguousarray(tgb)}
        )
    return in_maps


def kernel(input, target, gammas, _trace=False, _tmpdir=None):
    from concourse.bass_utils import run_bass_kernel_spmd

    in_maps = _make_in_maps(input, target, gammas, W)
    w = W
    if in_maps is None:
        # pathological target distribution: use full-width gather windows
        w = C
        in_maps = _make_in_maps(input, target, gammas, w)
        assert in_maps is not None  # w == C always satisfies the window check

    res = run_bass_kernel_spmd(
        _get_nc(w),
        in_maps,
        core_ids=list(range(NCORES)),
        trace=_trace,
        tmpdir=_tmpdir,
    )
    total = 0.0
    for i in range(NCORES):
        total += float(np.asarray(res.results[i]["out"], dtype=np.float64).sum())
    if _trace:
        kernel._last_result = res
    return np.array(-total, dtype=np.float32)


# revision 7
# speedup vs baseline: 1.0905x; 1.0022x over previous
"""AdaFocalLoss on 8 Trainium2 NeuronCores (Bass/Tile, SPMD).

Data-parallel over the batch axis: each core gets 8192 of the 65536
logit rows, the 15-entry gamma table is replicated, and the per-core
per-slot partial sums are combined on the host (the reduction over rows
is order-independent).

Per-core kernel structure (v2 — DMA-chunked):
  - Rows are assigned to (slot, partition) SORTED BY TARGET on the host:
    slot s holds the 128 rows whose targets sit near the s-th quantile of
    the target distribution, so all 128 targets of a slot fall inside a
    static 64-column window around the slot's quantile center.
  - The stream is issued as multi-slot chunk DMAs with a partition-major
    host layout, so each partition line is one 4*k*1000-byte descriptor
    (k slots/chunk).  Bigger descriptors cut the per-packet SDMA bus
    overhead (4KB lines run at ~345 GB/s; 16KB lines closer to the ~358
    GB/s HBM-per-core limit).  The chunk sizes taper: singles at the
    head (compute starts early), quads in steady state, and the final
    slot is two half-column DMAs so only ~0.6us of exp is exposed after
    the last byte lands.
  - ScalarE computes exp(x) for every element (fp16 out; the only
    engine with transcendentals).  The per-row sum of exps comes from
    the ACTIVATE's accum_out for 40 of the 64 slots and from a VectorE
    tensor_reduce over the exp tile for the other 24 — ScalarE ~87us
    and VectorE ~58us busy against the ~92us stream.
  - The target logit x_t is gathered on VectorE in one pass per slot:
    scalar_tensor_tensor (iota == (t - lo_s)) * x with accum_out over
    the slot's static 64-column window; the window iota and the
    per-slot-adjusted targets are host-precomputed constants (one small
    DMA — no [128,1000] iota constant).
  - Tail per row: lse = ln(sumexp), logpt = x_t - lse, pt = exp(logpt);
    gamma's sign s and magnitude m come from ONE fused telescope pass
    (ge = pt >= thr over a packed [ds|dm] delta table, multiply, reduce)
    instead of two; loss = -(1 + eps - s*pt)^m * logpt with the (1+eps,
    -1) affine folded into the Ln ACTIVATE.  The tail runs in four
    parts (32/24/7/1 slots) so only the 1-slot final part is exposed
    past the stream.
  - Each part writes its per-partition loss products into one column
    range of a [128, 64] tile; a single PE matmul against a ones vector
    reduces partitions, and the host sums the resulting [1, 64] f32
    outputs across cores (and negates).

The gather windows are data-independent quantile bands (+-32 columns
~ 5.8 sigma of the sampling deviation for iid targets; the reference
distribution measures a max deviation of 26).  If an unusual target
distribution ever falls outside them, the host check catches it and the
kernel transparently rebuilds with full-width windows (slower but
always correct).
"""

import sys

for _p in ("/opt/trn_rl_repo",):
    if _p not in sys.path:
        sys.path.insert(0, _p)

import numpy as np

NUM_BINS = 15
EPS = 1e-20
N, C = 65536, 1000
NCORES = 8
NSHARD = N // NCORES  # 8192 rows per core
P = 128  # SBUF partitions
R = NSHARD // P  # 64 row-slots per partition
W = 64  # gather window width (columns) per row-slot
ACT_SLOTS = 40  # row-sums accumulated on ScalarE (the rest on VectorE)
HALF_COL = 500  # column split point of the final slot's two DMAs


def _slot_lo(w):
    # static window starts: slot s is centered on the s-th target quantile
    return [min(max(int(C * (s + 0.5) / R) - w // 2, 0), C - w) for s in range(R)]


def _chunk_plan(w):
    """[(base_slot, n_slots), ...] + whether the last slot is halved.

    Multi-slot chunks need the partition-major host layout; the final
    halved slot overlaps its exp with its own DMA.  The full-width
    fallback (w == C) keeps every slot a single chunk because slot 63's
    gather window spans both halves there.
    """
    if w == C:
        return [(s, 1) for s in range(R)], False
    chunks = [(0, 1), (1, 1), (2, 2), (4, 4)]
    chunks += [(8 + 4 * i, 4) for i in range(12)]  # slots 8..55
    chunks += [(56, 2), (58, 2), (60, 2), (62, 1)]
    return chunks, True  # slot 63 in two half-column DMAs


# tail parts: only the small final part is exposed past the stream
TAIL_BOUNDS = [0, 32, 56, 63, 64]
NPART = len(TAIL_BOUNDS) - 1


def _split_excess_waits(nc, mybir, max_waits=1):
    """This container's walrus supports only one sync-wait command per
    instruction; hoist extra waits onto preceding same-engine no-ops."""
    ctr = 0
    for f in nc.m.functions:
        for bb in f.blocks:
            new_insts = []
            changed = False
            for inst in bb.instructions:
                si = inst.sync_info
                if si is not None and si.on_wait and len(si.on_wait) > max_waits:
                    waits = list(si.on_wait)
                    excess, keep = waits[:-max_waits], waits[-max_waits:]
                    for i in range(0, len(excess), max_waits):
                        ctr += 1
                        new_insts.append(
                            mybir.InstNoOp(
                                name=f"I-waitsplit-{ctr}",
                                sync_info=mybir.SyncInfo(
                                    on_wait=list(excess[i : i + max_waits]),
                                    on_update=[],
                                ),
                                bass_nofuse=True,
                                engine=inst.engine,
                            )
                        )
                    si.on_wait = keep
                    changed = True
                new_insts.append(inst)
            if changed:
                bb.instructions[:] = new_insts


def _build(w):
    import concourse.bass as bass
    import concourse.tile as tile
    from concourse import mybir

    f32 = mybir.dt.float32
    f16 = mybir.dt.float16
    AF = mybir.ActivationFunctionType
    ALU = mybir.AluOpType
    X = mybir.AxisListType.X
    NB = NUM_BINS
    slot_lo = _slot_lo(w)
    chunks, halved = _chunk_plan(w)

    nc = bass.Bass()
    x = nc.declare_dram_parameter("x", [NSHARD, C], f32, isOutput=False)
    # packed small constants: [tmap_adj (R) | gammas (NB) | iota (w)]
    tgb = nc.declare_dram_parameter("tgb", [P, R + NB + w], f32, isOutput=False)
    out = nc.declare_dram_parameter("out", [1, R], f32, isOutput=True)

    x_ap = x[:]

    # slots whose row-sum of exps is accumulated on ScalarE vs VectorE;
    # the late slots are all ScalarE so the exposed tail path is short
    n_d = R - ACT_SLOTS
    d_lim = 48 if w != C else R
    act_slots = set(range(R)) - set(
        s for s in range(d_lim) if (s * n_d) // d_lim != ((s + 1) * n_d) // d_lim
    )

    def slot_part(slot):
        h = 0
        while slot >= TAIL_BOUNDS[h + 1]:
            h += 1
        return h, slot - TAIL_BOUNDS[h]

    part_w = [TAIL_BOUNDS[h + 1] - TAIL_BOUNDS[h] for h in range(NPART)]

    with tile.TileContext(nc) as tc:
        with (
            tc.tile_pool(name="const", bufs=1) as cpool,
            tc.tile_pool(name="io", bufs=1) as iopool,
            tc.tile_pool(name="escr", bufs=3) as epool,
            tc.tile_pool(name="sscr", bufs=3) as spool,
            tc.tile_pool(name="acc", bufs=1) as apool,
            tc.tile_pool(name="tail", bufs=2) as tpool,
            tc.tile_pool(name="psum", bufs=1, space="PSUM") as ppool,
        ):
            # the first two chunks stream before the constant load so
            # compute can begin immediately
            def chunk_dma(ci):
                s0, k = chunks[ci]
                xt = iopool.tile(
                    [P, k * C], f32, tag=f"xt{k}", name=f"xtile_c{ci}", bufs=3 if k <= 1 else (3 if k == 4 else 2)
                )
                src = x_ap[s0 * P : (s0 + k) * P, :].rearrange(
                    "(p k) c -> p (k c)", p=P, k=k
                )
                nc.sync.dma_start(xt[:], src)
                return xt

            early = {ci: chunk_dma(ci) for ci in range(2)}

            tgb_t = cpool.tile([P, R + NB + w], f32, tag="tgb")
            nc.sync.dma_start(tgb_t[:], tgb[:])
            tmap_t = tgb_t[:, 0:R]
            gb_t = tgb_t[:, R : R + NB]
            iota_t = tgb_t[:, R + NB : R + NB + w]

            dsm = cpool.tile([P, 2 * NB], f32, tag="dsm")
            thr = cpool.tile([P, NB], f32, tag="thr")
            ones = cpool.tile([P, 1], f32, tag="ones")

            def derive_consts():
                # gamma sign/magnitude tables, telescoped into one packed
                # [ds | dm] delta table: g(bin(pt)) = sum_b dg_b*[pt>=b/15]
                sgn = cpool.tile([P, NB], f32, tag="sgn")
                nc.scalar.activation(sgn[:], gb_t, AF.Sign)
                mag = cpool.tile([P, NB], f32, tag="mag")
                nc.scalar.activation(mag[:], gb_t, AF.Abs)
                nc.vector.tensor_copy(dsm[:, 0:1], sgn[:, 0:1])
                nc.vector.tensor_sub(dsm[:, 1:NB], sgn[:, 1:NB], sgn[:, 0 : NB - 1])
                nc.vector.tensor_copy(dsm[:, NB : NB + 1], mag[:, 0:1])
                nc.vector.tensor_sub(
                    dsm[:, NB + 1 : 2 * NB], mag[:, 1:NB], mag[:, 0 : NB - 1]
                )
                # bin thresholds b/15, derived from the iota constant
                nc.vector.tensor_scalar(
                    thr[:], iota_t[:, 0:NB], 1.0 / NB, None, ALU.mult
                )
                nc.vector.memset(ones[:], 1.0)

            # per-part accumulators; the final part has two sumexp
            # columns when the last slot streams as two half DMAs
            se_w = list(part_w)
            if halved:
                se_w[-1] = 2
            sumexp = [
                apool.tile([P, se_w[h]], f32, tag=f"sumexp{h}", name=f"sumexp{h}")
                for h in range(NPART)
            ]
            xt_acc = [
                apool.tile([P, part_w[h]], f32, tag=f"xt{h}", name=f"xta{h}")
                for h in range(NPART)
            ]
            # per-slot per-partition loss products; one PE matmul reduces
            # partitions at the very end
            prodcat = apool.tile([P, R], f32, tag="prodcat")

            def gather(slot, xtile, off):
                h, col = slot_part(slot)
                lo = slot_lo[slot]
                so = spool.tile([P, w], f32, tag="so")
                nc.vector.scalar_tensor_tensor(
                    so[:],
                    iota_t,
                    tmap_t[:, slot : slot + 1],
                    xtile[:, off + lo : off + lo + w],
                    ALU.is_equal,
                    ALU.mult,
                    accum_out=xt_acc[h][:, col : col + 1],
                )

            def do_slot(slot, xtile, off):
                h, col = slot_part(slot)
                eo = epool.tile([P, C], f16, tag="eo")
                if slot in act_slots:
                    nc.scalar.activation(
                        eo[:],
                        xtile[:, off : off + C],
                        AF.Exp,
                        accum_out=sumexp[h][:, col : col + 1],
                    )
                else:
                    nc.scalar.activation(eo[:], xtile[:, off : off + C], AF.Exp)
                    nc.vector.tensor_reduce(
                        sumexp[h][:, col : col + 1], eo[:], X, ALU.add
                    )
                gather(slot, xtile, off)

            def tail_part(h):
                F = part_w[h]
                se = sumexp[h]
                if se_w[h] != F:  # halved final slot: combine the two sums
                    se2 = tpool.tile([P, 1], f32, tag="se2", name="se2")
                    nc.vector.tensor_add(se2[:], se[:, 0:1], se[:, 1:2])
                    se = se2
                lse = tpool.tile([P, F], f32, tag="lse")
                nc.scalar.activation(lse[:], se[:], AF.Ln)
                logpt = tpool.tile([P, F], f32, tag="logpt")
                nc.vector.tensor_sub(logpt[:], xt_acc[h][:], lse[:])
                pt = tpool.tile([P, F], f32, tag="pt")
                nc.scalar.activation(pt[:], logpt[:], AF.Exp)

                # fused telescope: ge[p,f,j,b] = pt[p,f] >= thr[p,b],
                # prods = ge * [ds|dm][p,j,b], reduce b -> sm[p,f,j]
                ge = tpool.tile([P, F * 2 * NB], f32, tag="ge")
                ge4 = ge[:].rearrange("p (f j b) -> p f j b", j=2, b=NB)
                pt_b = (
                    pt[:]
                    .rearrange("p (f j b) -> p f j b", j=1, b=1)
                    .broadcast_to([P, F, 2, NB])
                )
                thr_b = (
                    thr[:]
                    .rearrange("p (f j b) -> p f j b", f=1, j=1)
                    .broadcast_to([P, F, 2, NB])
                )
                nc.vector.tensor_tensor(ge4, pt_b, thr_b, ALU.is_ge)
                dsm_b = (
                    dsm[:]
                    .rearrange("p (f j b) -> p f j b", f=1, j=2)
                    .broadcast_to([P, F, 2, NB])
                )
                prods = tpool.tile([P, F * 2 * NB], f32, tag="prods")
                nc.vector.tensor_tensor(
                    prods[:].rearrange("p (f j b) -> p f j b", j=2, b=NB),
                    ge4,
                    dsm_b,
                    ALU.mult,
                )
                sm = tpool.tile([P, F * 2], f32, tag="sm")
                nc.vector.tensor_reduce(
                    sm[:], prods[:].rearrange("p (f j b) -> p f j b", j=2, b=NB),
                    X, ALU.add,
                )
                sm3 = sm[:].rearrange("p (f j) -> p f j", j=2)
                pt3 = pt[:].rearrange("p (f j) -> p f j", j=1)

                # u = 1 + eps - s*pt ; y = u^m = exp(m * ln(u)); the
                # (scale=-1, bias=1+eps) affine rides the Ln ACTIVATE
                spt = tpool.tile([P, F], f32, tag="spt")
                nc.vector.tensor_mul(
                    spt[:].rearrange("p (f j) -> p f j", j=1), sm3[:, :, 0:1], pt3
                )
                v = tpool.tile([P, F], f32, tag="v")
                nc.scalar.activation(v[:], spt[:], AF.Ln, bias=1.0 + EPS, scale=-1.0)
                w_ = tpool.tile([P, F], f32, tag="w")
                nc.vector.tensor_mul(
                    w_[:].rearrange("p (f j) -> p f j", j=1),
                    v[:].rearrange("p (f j) -> p f j", j=1),
                    sm3[:, :, 1:2],
                )
                y = tpool.tile([P, F], f32, tag="y")
                nc.scalar.activation(y[:], w_[:], AF.Exp)
                # per-slot per-partition partial of sum y*logpt (negated
                # on host)
                off = TAIL_BOUNDS[h]
                nc.vector.tensor_mul(prodcat[:, off : off + F], y[:], logpt[:])

            done_parts = set()
            for ci, (s0, k) in enumerate(chunks):
                xtile = early[ci] if ci in early else chunk_dma(ci)
                for j in range(k):
                    do_slot(s0 + j, xtile, j * C)
                if ci == 1:
                    # after the first two exps are queued, so the
                    # constant ACTIVATEs don't delay them on ScalarE
                    derive_consts()
                hi = s0 + k - 1
                for h in range(NPART - 1):
                    if hi >= TAIL_BOUNDS[h + 1] - 1 and h not in done_parts:
                        done_parts.add(h)
                        tail_part(h)  # overlaps the rest of the stream

            if halved:
                # final slot streams as two half-column DMAs: the first
                # half's exp overlaps the second half's transfer
                slot = R - 1
                lo = slot_lo[slot]
                for q, (c0, c1) in enumerate([(0, HALF_COL), (HALF_COL, C)]):
                    ht = iopool.tile(
                        [P, c1 - c0], f32, tag="xth", name=f"xth{q}", bufs=2
                    )
                    nc.sync.dma_start(ht[:], x_ap[slot * P : (slot + 1) * P, c0:c1])
                    eo = epool.tile([P, c1 - c0], f16, tag="eoh")
                    nc.scalar.activation(
                        eo[:], ht[:], AF.Exp,
                        accum_out=sumexp[NPART - 1][:, q : q + 1],
                    )
                    if c0 <= lo and lo + w <= c1:
                        so = spool.tile([P, w], f32, tag="so")
                        nc.vector.scalar_tensor_tensor(
                            so[:],
                            iota_t,
                            tmap_t[:, slot : slot + 1],
                            ht[:, lo - c0 : lo - c0 + w],
                            ALU.is_equal,
                            ALU.mult,
                            accum_out=xt_acc[NPART - 1][:, 0:1],
                        )
            tail_part(NPART - 1)

            ps = ppool.tile([1, R], f32, tag="ps")
            nc.tensor.matmul(ps[:], ones[:], prodcat[:], start=True, stop=True)
            res = tpool.tile([1, R], f32, tag="res")
            nc.scalar.copy(res[:], ps[:])
            nc.sync.dma_start(out[:], res[:])

    _split_excess_waits(nc, mybir, max_waits=1)
    return nc


_NC_CACHE = {}


def _get_nc(w):
    if w not in _NC_CACHE:
        _NC_CACHE[w] = _build(w)
    return _NC_CACHE[w]


def _make_in_maps(input, target, gammas, w):
    inp = np.ascontiguousarray(np.asarray(input, dtype=np.float32))
    tgt = np.asarray(target).astype(np.int64)
    gam = np.asarray(gammas, dtype=np.float32)
    assert inp.shape == (N, C) and tgt.shape == (N,) and gam.shape == (NUM_BINS,)

    slot_lo = np.asarray(_slot_lo(w), dtype=np.int64)
    chunks, _halved = _chunk_plan(w)

    in_maps = []
    for i in range(NCORES):
        tshard = tgt[NSHARD * i : NSHARD * (i + 1)]
        # sort rows by target; rank r -> slot r//P, partition r%P, so each
        # slot's 128 targets fall inside its static gather window
        order = np.argsort(tshard, kind="stable")
        tsorted = tshard[order]
        by_slot = tsorted.reshape(R, P)  # [slot, partition]
        lo = slot_lo[:, None]
        if not np.all((by_slot >= lo) & (by_slot <= lo + (w - 1))):
            return None  # caller falls back to full-width windows
        shard = inp[NSHARD * i : NSHARD * (i + 1)][order]
        # chunk layout: within a k-slot chunk, partition-major so each
        # partition line is one contiguous 4*k*C-byte DMA descriptor.
        # Slots outside the chunk list (the halved final slot) keep the
        # rank-major layout the copy starts from.
        xbuf = shard.copy()
        for s0, k in chunks:
            if k > 1:
                xbuf[s0 * P : (s0 + k) * P] = (
                    shard[s0 * P : (s0 + k) * P]
                    .reshape(k, P, C)
                    .transpose(1, 0, 2)
                    .reshape(k * P, C)
                )
        tmap_adj = (by_slot - lo).T.astype(np.float32)  # [P, R]
        tgb = np.concatenate(
            [
                tmap_adj,
                np.broadcast_to(gam, (P, NUM_BINS)),
                np.broadcast_to(np.arange(w, dtype=np.float32), (P, w)),
            ],
            axis=1,
        )
        in_maps.append(
            {"x": np.ascontiguousarray(xbuf), "tgb": np.ascontiguousarray(tgb)}
        )
    return in_maps


def kernel(input, target, gammas, _trace=False, _tmpdir=None):
    from concourse.bass_utils import run_bass_kernel_spmd

    in_maps = _make_in_maps(input, target, gammas, W)
    w = W
    if in_maps is None:
        # pathological target distribution: use full-width gather windows
        w = C
        in_maps = _make_in_maps(input, target, gammas, w)
        assert in_maps is not None  # w == C always satisfies the window check

    res = run_bass_kernel_spmd(
        _get_nc(w),
        in_maps,
        core_ids=list(range(NCORES)),
        trace=_trace,
        tmpdir=_tmpdir,
    )
    total = 0.0
    for i in range(NCORES):
        total += float(np.asarray(res.results[i]["out"], dtype=np.float64).sum())
    if _trace:
        kernel._last_result = res
    return np.array(-total, dtype=np.float32)


# revision 25
# speedup vs baseline: 1.0975x; 1.0063x over previous
"""AdaFocalLoss on 8 Trainium2 NeuronCores (Bass/Tile, SPMD).

Data-parallel over the batch axis: each core gets 8192 of the 65536
logit rows, the 15-entry gamma table is replicated, and the per-core
per-slot partial sums are combined on the host (the reduction over rows
is order-independent).

Per-core kernel structure (v6):
  - The 8192 rows form 64 (slot, partition) tiles of [128, 1000].  The
    stream is issued as multi-slot chunk DMAs with a partition-major
    host layout, so each partition line is one 4*k*1000-byte
    descriptor; 16KB descriptors measure ~400 GB/s HBM (4KB only ~345).
    Chunk sizes taper at both ends so compute starts early and little
    is exposed after the last byte.
  - ScalarE computes exp(x) (fp16 out; only engine with
    transcendentals).  Runs of slots whose row-sum goes to VectorE
    share ONE wide ACTIVATE (amortizes the fixed ~352-cycle cost); the
    remaining slots use per-slot ACTIVATEs whose accum_out produces the
    row-sum on ScalarE directly.  The split balances ScalarE (~74us)
    and VectorE (~73us) under the ~82us wire time.
  - The target logits are gathered by GpSimd indirect_copy (per 16-row
    partition group, index j of the group list = column idx[16q+j]):
    each chunk gathers [128, 16k] values of which the per-partition
    diagonal (i == p%16) is the true x_t; the tail extracts a whole
    part's diagonals with one broadcast multiply + reduce (~40ns/slot).
    No target sorting or window fallback is needed.
  - Tail per row: lse = ln(sumexp), logpt = x_t - lse, pt = exp(logpt);
    gamma's sign s and magnitude m come from one fused telescope pass
    (ge = pt >= thr over a packed [ds|dm] delta table, multiply,
    reduce); loss = -(1 + eps - s*pt)^m * logpt with the (1+eps, -1)
    affine folded into the Ln ACTIVATE.  Four parts (32/24/4/4 slots)
    so only the last part's serial chain is exposed past the stream.
  - Each part writes per-slot per-partition loss products into columns
    of a [128, 64] tile; one PE matmul against a ones vector reduces
    partitions and the host sums the [1, 64] outputs across cores (and
    negates).
"""

import sys

for _p in ("/opt/trn_rl_repo",):
    if _p not in sys.path:
        sys.path.insert(0, _p)

import numpy as np

NUM_BINS = 15
EPS = 1e-20
N, C = 65536, 1000
NCORES = 8
NSHARD = N // NCORES  # 8192 rows per core
P = 128  # SBUF partitions
R = NSHARD // P  # 64 row-slots per partition
G = 16  # indirect_copy gathers per-16-partition-group index lists

# chunk plan: tapered multi-slot DMAs (see module docstring)
CHUNKS = (
    [(0, 1), (1, 1), (2, 2), (4, 4)]
    + [(8 + 4 * i, 4) for i in range(12)]  # slots 8..55
    + [(56, 2), (58, 2), (60, 2), (62, 1), (63, 1)]
)
# slots whose row-sum comes from the ACTIVATE accum_out on ScalarE; the
# rest reduce on VectorE over the shared wide exp tile
A_SET = {0, 1} | set(range(50, 64))
# tail parts: only the small final part is exposed past the stream
TAIL_BOUNDS = [0, 32, 56, 60, 64]
NPART = len(TAIL_BOUNDS) - 1
NMETA = 2 * NUM_BINS + NUM_BINS + G  # [ds|dm] + thr + diag mask

# per-chunk start column in the index table, 2-aligned (the Q7 reads the
# u16 index lists as 4-byte words)
IDX_POS = []
_p = 0
for _s0, _k in CHUNKS:
    IDX_POS.append(_p)
    _p += _k + (_k & 1)
NIDX = _p


def _split_excess_waits(nc, mybir, max_waits=1):
    """This container's walrus supports only one sync-wait command per
    instruction; hoist extra waits onto preceding same-engine no-ops."""
    ctr = 0
    for f in nc.m.functions:
        for bb in f.blocks:
            new_insts = []
            changed = False
            for inst in bb.instructions:
                si = inst.sync_info
                if si is not None and si.on_wait and len(si.on_wait) > max_waits:
                    waits = list(si.on_wait)
                    excess, keep = waits[:-max_waits], waits[-max_waits:]
                    for i in range(0, len(excess), max_waits):
                        ctr += 1
                        new_insts.append(
                            mybir.InstNoOp(
                                name=f"I-waitsplit-{ctr}",
                                sync_info=mybir.SyncInfo(
                                    on_wait=list(excess[i : i + max_waits]),
                                    on_update=[],
                                ),
                                bass_nofuse=True,
                                engine=inst.engine,
                            )
                        )
                    si.on_wait = keep
                    changed = True
                new_insts.append(inst)
            if changed:
                bb.instructions[:] = new_insts


def _build():
    import concourse.bass as bass
    import concourse.tile as tile
    from concourse import mybir

    f32 = mybir.dt.float32
    f16 = mybir.dt.float16
    u16 = mybir.dt.uint16
    AF = mybir.ActivationFunctionType
    ALU = mybir.AluOpType
    X = mybir.AxisListType.X
    NB = NUM_BINS

    nc = bass.Bass()
    x = nc.declare_dram_parameter("x", [NSHARD, C], f32, isOutput=False)
    meta = nc.declare_dram_parameter("meta", [P, NMETA], f32, isOutput=False)
    idx = nc.declare_dram_parameter("idx", [P, NIDX], u16, isOutput=False)
    out = nc.declare_dram_parameter("out", [1, R], f32, isOutput=True)

    x_ap = x[:]

    def slot_part(slot):
        h = 0
        while slot >= TAIL_BOUNDS[h + 1]:
            h += 1
        return h, slot - TAIL_BOUNDS[h]

    part_w = [TAIL_BOUNDS[h + 1] - TAIL_BOUNDS[h] for h in range(NPART)]

    with tile.TileContext(nc) as tc:
        with (
            tc.tile_pool(name="const", bufs=1) as cpool,
            tc.tile_pool(name="io", bufs=1) as iopool,
            tc.tile_pool(name="escr", bufs=1) as epool,
            tc.tile_pool(name="acc", bufs=1) as apool,
            tc.tile_pool(name="tail", bufs=2) as tpool,
            tc.tile_pool(name="psum", bufs=1, space="PSUM") as ppool,
        ):
            def chunk_dma(ci):
                s0, k = CHUNKS[ci]
                xt = iopool.tile(
                    [P, k * C], f32, tag=f"xt{k}",
                    name=f"xtile_c{ci}",
                    bufs=3 if k in (1, 4) else 2,
                )
                src = x_ap[s0 * P : (s0 + k) * P, :].rearrange(
                    "(p k) c -> p (k c)", p=P, k=k
                )
                nc.sync.dma_start(xt[:], src)
                return xt

            # the first two chunks stream before the constant loads so
            # compute can begin immediately
            early = {ci: chunk_dma(ci) for ci in range(2)}

            meta_t = cpool.tile([P, NMETA], f32, tag="meta")
            nc.sync.dma_start(meta_t[:], meta[:])
            dsm = meta_t[:, 0 : 2 * NB]
            thr = meta_t[:, 2 * NB : 3 * NB]
            mask16 = meta_t[:, 3 * NB : 3 * NB + G]
            idx_t = cpool.tile([P, NIDX], u16, tag="idxt")
            nc.sync.dma_start(idx_t[:], idx[:])
            ones = cpool.tile([P, 1], f32, tag="ones")

            # per-slot accumulators: row-sums of exp, gathered candidate
            # values (diagonal = target logit), per-slot loss products
            sumexp = apool.tile([P, R], f32, tag="sumexp")
            xt_g = apool.tile([P, R * G], f32, tag="xt_g")
            prodcat = apool.tile([P, R], f32, tag="prodcat")

            def tail_part(h):
                F = part_w[h]
                off = TAIL_BOUNDS[h]
                # extract this part's target logits: per-partition
                # diagonal of the gathered [F, 16] groups
                gm = tpool.tile([P, F * G], f32, tag="gm", name=f"gm{h}")
                nc.vector.tensor_tensor(
                    gm[:].rearrange("p (f i) -> p f i", i=G),
                    xt_g[:, off * G : (off + F) * G].rearrange(
                        "p (f i) -> p f i", i=G
                    ),
                    mask16.rearrange("p (f i) -> p f i", f=1).broadcast_to(
                        [P, F, G]
                    ),
                    ALU.mult,
                )
                xt_p = tpool.tile([P, F], f32, tag="xt_p", name=f"xt_p{h}")
                nc.vector.tensor_reduce(
                    xt_p[:], gm[:].rearrange("p (f i) -> p f i", i=G), X, ALU.add
                )

                lse = tpool.tile([P, F], f32, tag="lse")
                nc.scalar.activation(lse[:], sumexp[:, off : off + F], AF.Ln)
                logpt = tpool.tile([P, F], f32, tag="logpt")
                nc.vector.tensor_sub(logpt[:], xt_p[:], lse[:])
                pt = tpool.tile([P, F], f32, tag="pt")
                nc.scalar.activation(pt[:], logpt[:], AF.Exp)

                # fused telescope: ge[p,f,j,b] = pt[p,f] >= thr[p,b],
                # prods = ge * [ds|dm][p,j,b], reduce b -> sm[p,f,j]
                ge = tpool.tile([P, F * 2 * NB], f32, tag="ge")
                ge4 = ge[:].rearrange("p (f j b) -> p f j b", j=2, b=NB)
                pt_b = (
                    pt[:]
                    .rearrange("p (f j b) -> p f j b", j=1, b=1)
                    .broadcast_to([P, F, 2, NB])
                )
                thr_b = thr.rearrange("p (f j b) -> p f j b", f=1, j=1).broadcast_to(
                    [P, F, 2, NB]
                )
                nc.vector.tensor_tensor(ge4, pt_b, thr_b, ALU.is_ge)
                dsm_b = dsm.rearrange("p (f j b) -> p f j b", f=1, j=2).broadcast_to(
                    [P, F, 2, NB]
                )
                prods = tpool.tile([P, F * 2 * NB], f32, tag="prods")
                nc.vector.tensor_tensor(
                    prods[:].rearrange("p (f j b) -> p f j b", j=2, b=NB),
                    ge4,
                    dsm_b,
                    ALU.mult,
                )
                sm = tpool.tile([P, F * 2], f32, tag="sm")
                nc.vector.tensor_reduce(
                    sm[:], prods[:].rearrange("p (f j b) -> p f j b", j=2, b=NB),
                    X, ALU.add,
                )
                sm3 = sm[:].rearrange("p (f j) -> p f j", j=2)
                pt3 = pt[:].rearrange("p (f j) -> p f j", j=1)

                # u = 1 + eps - s*pt ; y = u^m = exp(m * ln(u)); the
                # (scale=-1, bias=1+eps) affine rides the Ln ACTIVATE
                spt = tpool.tile([P, F], f32, tag="spt")
                nc.vector.tensor_mul(
                    spt[:].rearrange("p (f j) -> p f j", j=1), sm3[:, :, 0:1], pt3
                )
                v = tpool.tile([P, F], f32, tag="v")
                nc.scalar.activation(v[:], spt[:], AF.Ln, bias=1.0 + EPS, scale=-1.0)
                w_ = tpool.tile([P, F], f32, tag="w")
                nc.vector.tensor_mul(
                    w_[:].rearrange("p (f j) -> p f j", j=1),
                    v[:].rearrange("p (f j) -> p f j", j=1),
                    sm3[:, :, 1:2],
                )
                y = tpool.tile([P, F], f32, tag="y")
                nc.scalar.activation(y[:], w_[:], AF.Exp)
                # per-slot per-partition partial of sum y*logpt (negated
                # on host)
                nc.vector.tensor_mul(prodcat[:, off : off + F], y[:], logpt[:])

            done_parts = set()
            for ci, (s0, k) in enumerate(CHUNKS):
                xtile = early[ci] if ci in early else chunk_dma(ci)
                # gather the chunk's target-logit candidates on GpSimd
                ip = IDX_POS[ci]
                nc.gpsimd.indirect_copy(
                    xt_g[:, s0 * G : (s0 + k) * G],
                    xtile[:, 0 : k * C],
                    idx_t[:, ip : ip + k],
                    True,
                )
                # exps: runs of VectorE-summed slots share one wide
                # ACTIVATE; ScalarE-accum slots get their own
                j = 0
                while j < k:
                    s = s0 + j
                    if s in A_SET:
                        eo = epool.tile([P, C], f16, tag="eo", bufs=3)
                        nc.scalar.activation(
                            eo[:],
                            xtile[:, j * C : (j + 1) * C],
                            AF.Exp,
                            accum_out=sumexp[:, s : s + 1],
                        )
                        j += 1
                    else:
                        j2 = j
                        while j2 < k and (s0 + j2) not in A_SET:
                            j2 += 1
                        wdt = j2 - j
                        eow = epool.tile(
                            [P, wdt * C], f16, tag=f"eow{wdt}", bufs=3, name="eow"
                        )
                        nc.scalar.activation(
                            eow[:], xtile[:, j * C : j2 * C], AF.Exp
                        )
                        for jj in range(j, j2):
                            nc.vector.tensor_reduce(
                                sumexp[:, s0 + jj : s0 + jj + 1],
                                eow[:, (jj - j) * C : (jj - j + 1) * C],
                                X,
                                ALU.add,
                            )
                        j = j2
                if ci == 1:
                    nc.vector.memset(ones[:], 1.0)
                hi = s0 + k - 1
                for h in range(NPART):
                    if hi >= TAIL_BOUNDS[h + 1] - 1 and h not in done_parts:
                        done_parts.add(h)
                        tail_part(h)  # all but the last overlap the stream

            ps = ppool.tile([1, R], f32, tag="ps")
            nc.tensor.matmul(ps[:], ones[:], prodcat[:], start=True, stop=True)
            res = tpool.tile([1, R], f32, tag="res")
            nc.scalar.copy(res[:], ps[:])
            nc.sync.dma_start(out[:], res[:])

    _split_excess_waits(nc, mybir, max_waits=1)
    return nc


_NC_CACHE = {}


def _get_nc():
    if "nc" not in _NC_CACHE:
        _NC_CACHE["nc"] = _build()
    return _NC_CACHE["nc"]


def _make_in_maps(input, target, gammas):
    inp = np.ascontiguousarray(np.asarray(input, dtype=np.float32))
    tgt = np.asarray(target).astype(np.int64)
    gam = np.asarray(gammas, dtype=np.float32)
    assert inp.shape == (N, C) and tgt.shape == (N,) and gam.shape == (NUM_BINS,)

    # packed constants: telescoped [ds|dm] gamma deltas, bin thresholds,
    # and the per-partition diagonal mask for the grouped gather
    sgn, mag = np.sign(gam), np.abs(gam)
    ds = np.concatenate([sgn[:1], sgn[1:] - sgn[:-1]])
    dm = np.concatenate([mag[:1], mag[1:] - mag[:-1]])
    thr = np.arange(NUM_BINS, dtype=np.float32) / NUM_BINS
    mrow = np.concatenate([ds, dm, thr]).astype(np.float32)
    meta = np.zeros((P, NMETA), dtype=np.float32)
    meta[:, : 3 * NUM_BINS] = mrow
    meta[np.arange(P), 3 * NUM_BINS + (np.arange(P) % G)] = 1.0

    in_maps = []
    for i in range(NCORES):
        shard = inp[NSHARD * i : NSHARD * (i + 1)]
        tsh = tgt[NSHARD * i : NSHARD * (i + 1)].reshape(R, P)  # [slot, partition]
        # chunk layout: within a k-slot chunk, partition-major so each
        # partition line is one contiguous 4*k*C-byte DMA descriptor
        xbuf = shard.copy()
        for s0, k in CHUNKS:
            if k > 1:
                xbuf[s0 * P : (s0 + k) * P] = (
                    shard[s0 * P : (s0 + k) * P]
                    .reshape(k, P, C)
                    .transpose(1, 0, 2)
                    .reshape(k * P, C)
                )
        # gather indices: slot s (at position j of its chunk) gathers
        # column j*C + target
        off = np.empty(R, dtype=np.int64)
        for s0, k in CHUNKS:
            for j in range(k):
                off[s0 + j] = j * C
        idxv = np.zeros((P, NIDX), dtype=np.uint16)
        for ci, (s0, k) in enumerate(CHUNKS):
            ip = IDX_POS[ci]
            idxv[:, ip : ip + k] = tsh[s0 : s0 + k].T + off[None, s0 : s0 + k]
        in_maps.append(
            {
                "x": np.ascontiguousarray(xbuf),
                "meta": meta,
                "idx": np.ascontiguousarray(idxv),
            }
        )
    return in_maps


def kernel(input, target, gammas, _trace=False, _tmpdir=None):
    from concourse.bass_utils import run_bass_kernel_spmd

    in_maps = _make_in_maps(input, target, gammas)
    res = run_bass_kernel_spmd(
        _get_nc(),
        in_maps,
        core_ids=list(range(NCORES)),
        trace=_trace,
        tmpdir=_tmpdir,
    )
    total = 0.0
    for i in range(NCORES):
        total += float(np.asarray(res.results[i]["out"], dtype=np.float64).sum())
    if _trace:
        kernel._last_result = res
    return np.array(-total, dtype=np.float32)


# revision 26
# speedup vs baseline: 1.1365x; 1.0356x over previous
"""AdaFocalLoss on 8 Trainium2 NeuronCores (Bass/Tile, SPMD).

Data-parallel over the batch axis: each core gets 8192 of the 65536
logit rows, the 15-entry gamma table is replicated, and the per-core
per-slot partial sums are combined on the host (the reduction over rows
is order-independent).

Per-core kernel structure (v6):
  - The 8192 rows form 64 (slot, partition) tiles of [128, 1000].  The
    stream is issued as multi-slot chunk DMAs with a partition-major
    host layout, so each partition line is one 4*k*1000-byte
    descriptor; 16KB descriptors measure ~400 GB/s HBM (4KB only ~345).
    Chunk sizes taper at both ends so compute starts early and little
    is exposed after the last byte.
  - ScalarE computes exp(x) (fp16 out; only engine with
    transcendentals).  Runs of slots whose row-sum goes to VectorE
    share ONE wide ACTIVATE (amortizes the fixed ~352-cycle cost); the
    remaining slots use per-slot ACTIVATEs whose accum_out produces the
    row-sum on ScalarE directly.  The split balances ScalarE (~74us)
    and VectorE (~73us) under the ~82us wire time.
  - The target logits are gathered by GpSimd indirect_copy (per 16-row
    partition group, index j of the group list = column idx[16q+j]):
    each chunk gathers [128, 16k] values of which the per-partition
    diagonal (i == p%16) is the true x_t; the tail extracts a whole
    part's diagonals with one broadcast multiply + reduce (~40ns/slot).
    No target sorting or window fallback is needed.
  - Tail per row: lse = ln(sumexp), logpt = x_t - lse, pt = exp(logpt);
    gamma's sign s and magnitude m come from one fused telescope pass
    (ge = pt >= thr over a packed [ds|dm] delta table, multiply,
    reduce); loss = -(1 + eps - s*pt)^m * logpt with the (1+eps, -1)
    affine folded into the Ln ACTIVATE.  Four parts (32/24/4/4 slots)
    so only the last part's serial chain is exposed past the stream.
  - Each part writes per-slot per-partition loss products into columns
    of a [128, 64] tile; one PE matmul against a ones vector reduces
    partitions and the host sums the [1, 64] outputs across cores (and
    negates).
"""

import sys

for _p in ("/opt/trn_rl_repo",):
    if _p not in sys.path:
        sys.path.insert(0, _p)

import numpy as np

NUM_BINS = 15
EPS = 1e-20
N, C = 65536, 1000
NCORES = 8
NSHARD = N // NCORES  # 8192 rows per core
P = 128  # SBUF partitions
R = NSHARD // P  # 64 row-slots per partition
G = 16  # indirect_copy gathers per-16-partition-group index lists

# chunk plan: tapered multi-slot DMAs (see module docstring)
CHUNKS = (
    [(0, 1), (1, 1), (2, 2), (4, 4)]
    + [(8 + 4 * i, 4) for i in range(12)]  # slots 8..55
    + [(56, 2), (58, 2), (60, 2), (62, 1), (63, 1)]
)
# slots whose row-sum comes from the ACTIVATE accum_out on ScalarE; the
# rest reduce on VectorE over the shared wide exp tile.  Mid-stream
# quads host most of them (the stream tail stays wide so ScalarE can
# race the wire there); the final slots take the short RD_ACC path.
A_SET = (
    {0, 1}
    | {s for q in range(8, 36, 4) for s in (q, q + 1)}
    | set(range(58, 64))
)
# tail parts: only the small final part is exposed past the stream
TAIL_BOUNDS = [0, 32, 56, 60, 64]
NPART = len(TAIL_BOUNDS) - 1
NMETA = 2 * NUM_BINS + NUM_BINS + G  # [ds|dm] + thr + diag mask

# per-chunk start column in the index table, 2-aligned (the Q7 reads the
# u16 index lists as 4-byte words)
IDX_POS = []
_p = 0
for _s0, _k in CHUNKS:
    IDX_POS.append(_p)
    _p += _k + (_k & 1)
NIDX = _p


def _split_excess_waits(nc, mybir, max_waits=1):
    """This container's walrus supports only one sync-wait command per
    instruction; hoist extra waits onto preceding same-engine no-ops."""
    ctr = 0
    for f in nc.m.functions:
        for bb in f.blocks:
            new_insts = []
            changed = False
            for inst in bb.instructions:
                si = inst.sync_info
                if si is not None and si.on_wait and len(si.on_wait) > max_waits:
                    waits = list(si.on_wait)
                    excess, keep = waits[:-max_waits], waits[-max_waits:]
                    for i in range(0, len(excess), max_waits):
                        ctr += 1
                        new_insts.append(
                            mybir.InstNoOp(
                                name=f"I-waitsplit-{ctr}",
                                sync_info=mybir.SyncInfo(
                                    on_wait=list(excess[i : i + max_waits]),
                                    on_update=[],
                                ),
                                bass_nofuse=True,
                                engine=inst.engine,
                            )
                        )
                    si.on_wait = keep
                    changed = True
                new_insts.append(inst)
            if changed:
                bb.instructions[:] = new_insts


def _build():
    import concourse.bass as bass
    import concourse.tile as tile
    from concourse import mybir

    f32 = mybir.dt.float32
    f16 = mybir.dt.float16
    u16 = mybir.dt.uint16
    AF = mybir.ActivationFunctionType
    ALU = mybir.AluOpType
    X = mybir.AxisListType.X
    NB = NUM_BINS

    nc = bass.Bass()
    x = nc.declare_dram_parameter("x", [NSHARD, C], f32, isOutput=False)
    meta = nc.declare_dram_parameter("meta", [P, NMETA], f32, isOutput=False)
    idx = nc.declare_dram_parameter("idx", [P, NIDX], u16, isOutput=False)
    out = nc.declare_dram_parameter("out", [1, R], f32, isOutput=True)

    x_ap = x[:]

    def slot_part(slot):
        h = 0
        while slot >= TAIL_BOUNDS[h + 1]:
            h += 1
        return h, slot - TAIL_BOUNDS[h]

    part_w = [TAIL_BOUNDS[h + 1] - TAIL_BOUNDS[h] for h in range(NPART)]

    with tile.TileContext(nc) as tc:
        with (
            tc.tile_pool(name="const", bufs=1) as cpool,
            tc.tile_pool(name="io", bufs=1) as iopool,
            tc.tile_pool(name="escr", bufs=1) as epool,
            tc.tile_pool(name="acc", bufs=1) as apool,
            tc.tile_pool(name="tail", bufs=2) as tpool,
            tc.tile_pool(name="psum", bufs=1, space="PSUM") as ppool,
        ):
            def chunk_dma(ci):
                s0, k = CHUNKS[ci]
                xt = iopool.tile(
                    [P, k * C], f32, tag=f"xt{k}",
                    name=f"xtile_c{ci}",
                    bufs=3 if k in (1, 4) else 2,
                )
                src = x_ap[s0 * P : (s0 + k) * P, :].rearrange(
                    "(p k) c -> p (k c)", p=P, k=k
                )
                nc.sync.dma_start(xt[:], src)
                return xt

            # the first two chunks stream before the constant loads so
            # compute can begin immediately
            early = {ci: chunk_dma(ci) for ci in range(2)}

            meta_t = cpool.tile([P, NMETA], f32, tag="meta")
            nc.sync.dma_start(meta_t[:], meta[:])
            dsm = meta_t[:, 0 : 2 * NB]
            thr = meta_t[:, 2 * NB : 3 * NB]
            mask16 = meta_t[:, 3 * NB : 3 * NB + G]
            idx_t = cpool.tile([P, NIDX], u16, tag="idxt")
            nc.sync.dma_start(idx_t[:], idx[:])
            ones = cpool.tile([P, 1], f32, tag="ones")

            # per-slot accumulators: row-sums of exp, gathered candidate
            # values (diagonal = target logit), per-slot loss products
            sumexp = apool.tile([P, R], f32, tag="sumexp")
            xt_g = apool.tile([P, R * G], f32, tag="xt_g")
            prodcat = apool.tile([P, R], f32, tag="prodcat")

            def tail_part(h):
                F = part_w[h]
                off = TAIL_BOUNDS[h]
                # extract this part's target logits: per-partition
                # diagonal of the gathered [F, 16] groups
                gm = tpool.tile([P, F * G], f32, tag="gm", name=f"gm{h}")
                nc.vector.tensor_tensor(
                    gm[:].rearrange("p (f i) -> p f i", i=G),
                    xt_g[:, off * G : (off + F) * G].rearrange(
                        "p (f i) -> p f i", i=G
                    ),
                    mask16.rearrange("p (f i) -> p f i", f=1).broadcast_to(
                        [P, F, G]
                    ),
                    ALU.mult,
                )
                xt_p = tpool.tile([P, F], f32, tag="xt_p", name=f"xt_p{h}")
                nc.vector.tensor_reduce(
                    xt_p[:], gm[:].rearrange("p (f i) -> p f i", i=G), X, ALU.add
                )

                lse = tpool.tile([P, F], f32, tag="lse")
                nc.scalar.activation(lse[:], sumexp[:, off : off + F], AF.Ln)
                logpt = tpool.tile([P, F], f32, tag="logpt")
                nc.vector.tensor_sub(logpt[:], xt_p[:], lse[:])
                pt = tpool.tile([P, F], f32, tag="pt")
                nc.scalar.activation(pt[:], logpt[:], AF.Exp)

                # fused telescope: ge[p,f,j,b] = pt[p,f] >= thr[p,b],
                # prods = ge * [ds|dm][p,j,b], reduce b -> sm[p,f,j]
                ge = tpool.tile([P, F * 2 * NB], f32, tag="ge")
                ge4 = ge[:].rearrange("p (f j b) -> p f j b", j=2, b=NB)
                pt_b = (
                    pt[:]
                    .rearrange("p (f j b) -> p f j b", j=1, b=1)
                    .broadcast_to([P, F, 2, NB])
                )
                thr_b = thr.rearrange("p (f j b) -> p f j b", f=1, j=1).broadcast_to(
                    [P, F, 2, NB]
                )
                nc.vector.tensor_tensor(ge4, pt_b, thr_b, ALU.is_ge)
                dsm_b = dsm.rearrange("p (f j b) -> p f j b", f=1, j=2).broadcast_to(
                    [P, F, 2, NB]
                )
                prods = tpool.tile([P, F * 2 * NB], f32, tag="prods")
                nc.vector.tensor_tensor(
                    prods[:].rearrange("p (f j b) -> p f j b", j=2, b=NB),
                    ge4,
                    dsm_b,
                    ALU.mult,
                )
                sm = tpool.tile([P, F * 2], f32, tag="sm")
                nc.vector.tensor_reduce(
                    sm[:], prods[:].rearrange("p (f j b) -> p f j b", j=2, b=NB),
                    X, ALU.add,
                )
                sm3 = sm[:].rearrange("p (f j) -> p f j", j=2)
                pt3 = pt[:].rearrange("p (f j) -> p f j", j=1)

                # u = 1 + eps - s*pt ; y = u^m = exp(m * ln(u)); the
                # (scale=-1, bias=1+eps) affine rides the Ln ACTIVATE
                spt = tpool.tile([P, F], f32, tag="spt")
                nc.vector.tensor_mul(
                    spt[:].rearrange("p (f j) -> p f j", j=1), sm3[:, :, 0:1], pt3
                )
                v = tpool.tile([P, F], f32, tag="v")
                nc.scalar.activation(v[:], spt[:], AF.Ln, bias=1.0 + EPS, scale=-1.0)
                w_ = tpool.tile([P, F], f32, tag="w")
                nc.vector.tensor_mul(
                    w_[:].rearrange("p (f j) -> p f j", j=1),
                    v[:].rearrange("p (f j) -> p f j", j=1),
                    sm3[:, :, 1:2],
                )
                y = tpool.tile([P, F], f32, tag="y")
                nc.scalar.activation(y[:], w_[:], AF.Exp)
                # per-slot per-partition partial of sum y*logpt (negated
                # on host)
                nc.vector.tensor_mul(prodcat[:, off : off + F], y[:], logpt[:])

            done_parts = set()
            for ci, (s0, k) in enumerate(CHUNKS):
                xtile = early[ci] if ci in early else chunk_dma(ci)
                # gather the chunk's target-logit candidates on GpSimd
                ip = IDX_POS[ci]
                nc.gpsimd.indirect_copy(
                    xt_g[:, s0 * G : (s0 + k) * G],
                    xtile[:, 0 : k * C],
                    idx_t[:, ip : ip + k],
                    True,
                )
                # exps: runs of VectorE-summed slots share one wide
                # ACTIVATE; ScalarE-accum slots get their own
                j = 0
                while j < k:
                    s = s0 + j
                    if s in A_SET:
                        eo = epool.tile([P, C], f16, tag="eo", bufs=3)
                        nc.scalar.activation(
                            eo[:],
                            xtile[:, j * C : (j + 1) * C],
                            AF.Exp,
                            accum_out=sumexp[:, s : s + 1],
                        )
                        j += 1
                    else:
                        j2 = j
                        while j2 < k and (s0 + j2) not in A_SET:
                            j2 += 1
                        wdt = j2 - j
                        eow = epool.tile(
                            [P, wdt * C], f16, tag=f"eow{wdt}", bufs=3, name="eow"
                        )
                        nc.scalar.activation(
                            eow[:], xtile[:, j * C : j2 * C], AF.Exp
                        )
                        for jj in range(j, j2):
                            nc.vector.tensor_reduce(
                                sumexp[:, s0 + jj : s0 + jj + 1],
                                eow[:, (jj - j) * C : (jj - j + 1) * C],
                                X,
                                ALU.add,
                            )
                        j = j2
                if ci == 1:
                    nc.vector.memset(ones[:], 1.0)
                hi = s0 + k - 1
                for h in range(NPART):
                    if hi >= TAIL_BOUNDS[h + 1] - 1 and h not in done_parts:
                        done_parts.add(h)
                        tail_part(h)  # all but the last overlap the stream

            ps = ppool.tile([1, R], f32, tag="ps")
            nc.tensor.matmul(ps[:], ones[:], prodcat[:], start=True, stop=True)
            res = tpool.tile([1, R], f32, tag="res")
            nc.scalar.copy(res[:], ps[:])
            nc.sync.dma_start(out[:], res[:])

    _split_excess_waits(nc, mybir, max_waits=1)
    return nc


_NC_CACHE = {}


def _get_nc():
    if "nc" not in _NC_CACHE:
        _NC_CACHE["nc"] = _build()
    return _NC_CACHE["nc"]


def _make_in_maps(input, target, gammas):
    inp = np.ascontiguousarray(np.asarray(input, dtype=np.float32))
    tgt = np.asarray(target).astype(np.int64)
    gam = np.asarray(gammas, dtype=np.float32)
    assert inp.shape == (N, C) and tgt.shape == (N,) and gam.shape == (NUM_BINS,)

    # packed constants: telescoped [ds|dm] gamma deltas, bin thresholds,
    # and the per-partition diagonal mask for the grouped gather
    sgn, mag = np.sign(gam), np.abs(gam)
    ds = np.concatenate([sgn[:1], sgn[1:] - sgn[:-1]])
    dm = np.concatenate([mag[:1], mag[1:] - mag[:-1]])
    thr = np.arange(NUM_BINS, dtype=np.float32) / NUM_BINS
    mrow = np.concatenate([ds, dm, thr]).astype(np.float32)
    meta = np.zeros((P, NMETA), dtype=np.float32)
    meta[:, : 3 * NUM_BINS] = mrow
    meta[np.arange(P), 3 * NUM_BINS + (np.arange(P) % G)] = 1.0

    in_maps = []
    for i in range(NCORES):
        shard = inp[NSHARD * i : NSHARD * (i + 1)]
        tsh = tgt[NSHARD * i : NSHARD * (i + 1)].reshape(R, P)  # [slot, partition]
        # chunk layout: within a k-slot chunk, partition-major so each
        # partition line is one contiguous 4*k*C-byte DMA descriptor
        xbuf = shard.copy()
        for s0, k in CHUNKS:
            if k > 1:
                xbuf[s0 * P : (s0 + k) * P] = (
                    shard[s0 * P : (s0 + k) * P]
                    .reshape(k, P, C)
                    .transpose(1, 0, 2)
                    .reshape(k * P, C)
                )
        # gather indices: slot s (at position j of its chunk) gathers
        # column j*C + target
        off = np.empty(R, dtype=np.int64)
        for s0, k in CHUNKS:
            for j in range(k):
                off[s0 + j] = j * C
        idxv = np.zeros((P, NIDX), dtype=np.uint16)
        for ci, (s0, k) in enumerate(CHUNKS):
            ip = IDX_POS[ci]
            idxv[:, ip : ip + k] = tsh[s0 : s0 + k].T + off[None, s0 : s0 + k]
        in_maps.append(
            {
                "x": np.ascontiguousarray(xbuf),
                "meta": meta,
                "idx": np.ascontiguousarray(idxv),
            }
        )
    return in_maps


def kernel(input, target, gammas, _trace=False, _tmpdir=None):
    from concourse.bass_utils import run_bass_kernel_spmd

    in_maps = _make_in_maps(input, target, gammas)
    res = run_bass_kernel_spmd(
        _get_nc(),
        in_maps,
        core_ids=list(range(NCORES)),
        trace=_trace,
        tmpdir=_tmpdir,
    )
    total = 0.0
    for i in range(NCORES):
        total += float(np.asarray(res.results[i]["out"], dtype=np.float64).sum())
    if _trace:
        kernel._last_result = res
    return np.array(-total, dtype=np.float32)
